# revision 1
# baseline (speedup 1.0000x reference)
"""DKEPooling Trainium2 kernel.

Per-graph pipeline (d=256, n=512 nodes/graph):
  f = feat + 0.01*noise
  C' = f^T f - colsum(f)^T colssum(f)/n          (= (n-1)*cov, Gram + rank-1 PSUM trick)
  A  = C'/tr(C')
  Newton-Schulz (5 iter) reformulated via the commuting-polynomial invariant
  T_k := A Z_k^2:  T_{k+1} = 0.25 T_k (3I - T_k)^2   -> only 6 d^3 matmuls/graph
  (A^2, then 2 per T-step), followed by an 8-matvec tail applied to the mean
  (all remaining NS factors are applied vector-wise, never materialized).

Sharding: data-parallel over graphs. 8 cores x 16 graphs; no cross-core comm.
"""
import numpy as np

import concourse.bacc as bacc
import concourse.bass as bass
import concourse.mybir as mybir
import concourse.tile as tile
from concourse.bass_utils import run_bass_kernel_spmd

F32 = mybir.dt.float32
BF16 = mybir.dt.bfloat16
F32R = mybir.dt.float32r
ALU = mybir.AluOpType
ACTF = mybir.ActivationFunctionType

N_CORES = 8
D = 256
NPG = 512
B_TOTAL = 128
B_CORE = B_TOTAL // N_CORES      # 16 graphs per core
ROWS_CORE = B_CORE * NPG         # 8192 feat rows per core
W = 4                            # graphs per tail wave
N_WAVES = B_CORE // W

# const tensor layout (f32 [128, 772]):
#   [:, 0:256]   = [3I | 0]   (3I block for row-chunk 0)
#   [:, 256:512] = [0 | 3I]   (3I block for row-chunk 1)
#   [:, 512:640] = I128
#   [:, 640]     = ones column
#   [0, 641:769] = ones row
CST_COLS = 772


def _const_arrays():
    import ml_dtypes
    cst = np.zeros((128, CST_COLS), np.float32)
    eye = np.eye(128, dtype=np.float32)
    cst[:, 0:128] = 3.0 * eye
    cst[:, 384:512] = 3.0 * eye
    cst[:, 512:640] = eye
    cst[:, 640] = 1.0
    cst[0, 641:769] = 1.0
    cstb = np.ones((128, 1), ml_dtypes.bfloat16)
    cstr = np.eye(W, dtype=np.float32)
    return cst, cstb, cstr


def _r(ap):
    return ap.bitcast(F32R)


def build_module():
    nc = bacc.Bacc(None, target_bir_lowering=False)
    feat_d = nc.declare_dram_parameter("feat", [ROWS_CORE, D], F32, isOutput=False)
    noise_d = nc.declare_dram_parameter("noise", [ROWS_CORE, D], F32, isOutput=False)
    cst_d = nc.declare_dram_parameter("cst", [128, CST_COLS], F32, isOutput=False)
    cstb_d = nc.declare_dram_parameter("cstb", [128, 1], BF16, isOutput=False)
    cstr_d = nc.declare_dram_parameter("cstr", [W, W], F32R, isOutput=False)
    out_d = nc.declare_dram_parameter("out", [B_CORE, D], F32, isOutput=True)

    with tile.TileContext(nc) as tc:
        _build_tile(tc, nc, feat_d, noise_d, cst_d, cstb_d, cstr_d, out_d)
    nc.compile()
    return nc


def _build_tile(tc, nc, feat_d, noise_d, cst_d, cstb_d, cstr_d, out_d):
    import contextlib
    ctx = contextlib.ExitStack()
    with ctx:
        stage_p = ctx.enter_context(tc.tile_pool(name="stage", bufs=5))
        g_p = ctx.enter_context(tc.tile_pool(name="gp", bufs=6))
        mats_p = ctx.enter_context(tc.tile_pool(name="mats", bufs=7))
        chain_p = ctx.enter_context(tc.tile_pool(name="chain", bufs=3))
        small_p = ctx.enter_context(tc.tile_pool(name="small", bufs=6))
        rows_p = ctx.enter_context(tc.tile_pool(name="rows", bufs=3))
        tail_p = ctx.enter_context(tc.tile_pool(name="tailp", bufs=3))
        cst_p = ctx.enter_context(tc.tile_pool(name="cstp", bufs=1))
        psG = ctx.enter_context(tc.tile_pool(name="psG", bufs=3, space="PSUM"))
        psS = ctx.enter_context(tc.tile_pool(name="psS", bufs=1, space="PSUM"))
        psUR = ctx.enter_context(tc.tile_pool(name="psUR", bufs=2, space="PSUM"))
        psT = ctx.enter_context(tc.tile_pool(name="psT", bufs=2, space="PSUM"))

        cst = cst_p.tile([128, CST_COLS], F32, tag="cst", name="cst_sb")
        nc.gpsimd.dma_start(cst, cst_d[:, :])
        onesb = cst_p.tile([128, 1], BF16, tag="onesb", name="onesb_sb")
        nc.gpsimd.dma_start(onesb, cstb_d[:, :])
        IWr = cst_p.tile([W, W], F32R, tag="iwr", name="iwr_sb")
        nc.gpsimd.dma_start(IWr, cstr_d[:, :])

        def c3I(m):
            return cst[:, 256 * m:256 * (m + 1)]

        I128 = cst[:, 512:640]
        ones_col = cst[:, 640:641]
        ones_row = cst[0:1, 641:769]

        def phase_a(g, V0ROWS, b):
            """Load graph g, compute A and T1..T3; returns dict of kept tiles."""
            # One big [128, 4*256] tile per tensor: the graph's 512 rows as 4
            # row-chunks side by side in the free dim; a single SWDGE DMA each
            # (HWDGE fans one transfer across several queue semaphores, which
            # overflows the DVE consumers' wait slots).
            ft = stage_p.tile([128, 4 * D], F32, tag="ft", name=f"ft_{g}")
            nc.gpsimd.dma_start(
                ft, feat_d[g * NPG:(g + 1) * NPG, :].rearrange("(c p) d -> p c d", p=128))
            nz = stage_p.tile([128, 4 * D], F32, tag="nz", name=f"nz_{g}")
            nc.gpsimd.dma_start(
                nz, noise_d[g * NPG:(g + 1) * NPG, :].rearrange("(c p) d -> p c d", p=128))
            gb = g_p.tile([128, 4 * D], BF16, tag="g", name=f"g_{g}")
            # f = (noise * 0.01) + feat, rounded to bf16 for the Gram.
            # Keep the DVE arithmetic in plain f32 (in-place, standard ISA
            # structs with enough sync slots) and convert to bf16 on ACT;
            # dtype-converting DVE ops lower to custom ucode with too few
            # sync-wait slots for walrus.
            nc.vector.scalar_tensor_tensor(gb, nz, 0.01, ft, ALU.mult, ALU.add)
            gt = [gb[:, k * D:(k + 1) * D] for k in range(4)]

            # Gram into PSUM: G_m = sum_k g_k[:, m*128:...].T @ g_k   (stop on corr MM)
            G = [psG.tile([128, D], F32, tag="G", name=f"G{m}_{g}") for m in range(2)]
            for k in range(4):
                for m in range(2):
                    nc.tensor.matmul(G[m], gt[k][:, m * 128:(m + 1) * 128], gt[k],
                                     start=(k == 0), stop=False)
            # column sums s = ones^T g
            s_ps = psS.tile([1, D], F32, tag="ps_small", name=f"s_{g}")
            for k in range(4):
                nc.tensor.matmul(s_ps, onesb, gt[k], start=(k == 0), stop=(k == 3))
            srow = small_p.tile([1, D], BF16, tag="srow", name=f"srow_{g}")
            nc.scalar.copy(srow, s_ps)
            srow_n = small_p.tile([1, D], BF16, tag="srow_n", name=f"srown_{g}")
            nc.vector.tensor_scalar_mul(srow_n, srow, -1.0 / NPG)

            # rank-1 mean correction accumulated into the Gram PSUM:
            # C' = G - s^T s / n
            for m in range(2):
                nc.tensor.matmul(G[m], srow_n[0:1, m * 128:(m + 1) * 128], srow,
                                 start=False, stop=True)

            # Evacuate C' from PSUM via ACT (DVE reads of PSUM crash the
            # exec unit on this runtime; ACT reads are fine).
            Gc = []
            for m in range(2):
                gc = chain_p.tile([128, D], F32, tag=f"Gc{m}", name=f"Gc{m}_{g}")
                nc.scalar.copy(gc, G[m])
                Gc.append(gc)
            # trace via diag mask + GPSIMD partition all-reduce (the tiny
            # fp32 PE matmuls this used before crash the exec unit)
            scr = small_p.tile([128, 128], F32, tag="scr", name=f"scr_{g}")
            dg = small_p.tile([128, 2], F32, tag="dg", name=f"dg_{g}")
            for m in range(2):
                nc.vector.scalar_tensor_tensor(scr, Gc[m][:, m * 128:(m + 1) * 128],
                                               1.0, I128, ALU.mult, ALU.mult,
                                               accum_out=dg[:, m:m + 1])
            import concourse.bass_isa as bass_isa
            dgs = small_p.tile([128, 1], F32, tag="dgs", name=f"dgs_{g}")
            nc.vector.tensor_add(dgs, dg[:, 0:1], dg[:, 1:2])
            trc = small_p.tile([128, 1], F32, tag="trc", name=f"trc_{g}")
            nc.gpsimd.partition_all_reduce(trc, dgs, 128, bass_isa.ReduceOp.add)
            rcpb = small_p.tile([128, 1], F32, tag="rcpb", name=f"rcpb_{g}")
            nc.vector.reciprocal(rcpb, trc)
            sq = small_p.tile([1, 1], F32, tag="sq", name=f"sq_{g}")
            nc.scalar.activation(sq, trc[0:1, 0:1], ACTF.Sqrt, scale=1.0 / (NPG - 1))
            cb = small_p.tile([1, 1], F32, tag="cb", name=f"cb_{g}")
            nc.vector.tensor_scalar_mul(cb, sq, 0.03125 / NPG)
            # v0 row for the tail: mean scaled by all folded constants.
            # Computed at partition 0, DMA'd into row b of V0ROWS (compute
            # engines cannot write non-32-aligned partition bases).
            v0r = small_p.tile([1, D], F32R, tag="v0r", name=f"v0r_{g}")
            nc.scalar.activation(v0r, s_ps, ACTF.Copy, scale=cb)
            nc.sync.dma_start(V0ROWS[b:b + 1, :], v0r)

            A = []
            for m in range(2):
                Am = mats_p.tile([128, D], F32R, tag=f"A{m}", name=f"A{m}_{g}")
                nc.vector.tensor_scalar_mul(Am, Gc[m], rcpb)
                A.append(Am)

            def mm256(tag, L, R, dst_pool, dst_tag):
                dst = [dst_pool.tile([128, D], F32, tag=dst_tag, name=f"{tag}{m}_{g}")
                       for m in range(2)]
                for m in range(2):
                    for k in range(2):
                        nc.tensor.matmul(dst[m], L[k][:, m * 128:(m + 1) * 128],
                                         R[k], start=(k == 0), stop=(k == 1))
                return dst

            # T-chain: A2 -> T1 -> T2 -> T3 (2 matmuls per step after A2)
            A2 = mm256("A2", A, A, psG, "G")
            W1 = []
            V0 = []
            for m in range(2):
                a2c = chain_p.tile([128, D], F32, tag=f"A2c{m}", name=f"A2c{m}_{g}")
                nc.scalar.copy(a2c, A2[m])
                w1 = chain_p.tile([128, D], F32R, tag=f"W1{m}", name=f"W1{m}_{g}")
                nc.vector.scalar_tensor_tensor(w1, A[m], 3.0, a2c, ALU.mult, ALU.subtract)
                W1.append(w1)
                v0 = chain_p.tile([128, D], F32R, tag=f"V0{m}", name=f"V0{m}_{g}")
                nc.vector.scalar_tensor_tensor(v0, A[m], -1.0, c3I(m), ALU.mult, ALU.add)
                V0.append(v0)
            P = mm256("P", W1, V0, psG, "G")
            T1 = []
            V1 = []
            for m in range(2):
                t1 = mats_p.tile([128, D], F32R, tag=f"T1{m}", name=f"T1{m}_{g}")
                nc.scalar.mul(t1, P[m], 0.25)
                T1.append(t1)
                v1 = chain_p.tile([128, D], F32R, tag=f"V1{m}", name=f"V1{m}_{g}")
                nc.vector.scalar_tensor_tensor(v1, t1, -1.0, c3I(m), ALU.mult, ALU.add)
                V1.append(v1)
            Q = mm256("Q", T1, V1, psG, "G")
            Qb = []
            for m in range(2):
                qb = chain_p.tile([128, D], F32R, tag=f"Qb{m}", name=f"Qb{m}_{g}")
                nc.scalar.copy(qb, Q[m])
                Qb.append(qb)
            R = mm256("R", Qb, V1, psG, "G")
            T2 = []
            V2 = []
            for m in range(2):
                t2 = mats_p.tile([128, D], F32R, tag=f"T2{m}", name=f"T2{m}_{g}")
                nc.scalar.mul(t2, R[m], 0.25)
                T2.append(t2)
                v2 = chain_p.tile([128, D], F32R, tag=f"V2{m}", name=f"V2{m}_{g}")
                nc.vector.scalar_tensor_tensor(v2, t2, -1.0, c3I(m), ALU.mult, ALU.add)
                V2.append(v2)
            S = mm256("S", T2, V2, psG, "G")
            Sb = []
            for m in range(2):
                sb_ = chain_p.tile([128, D], F32R, tag=f"Sb{m}", name=f"Sb{m}_{g}")
                nc.scalar.copy(sb_, S[m])
                Sb.append(sb_)
            U = mm256("U", Sb, V2, psG, "G")
            T3 = []
            for m in range(2):
                t3 = mats_p.tile([128, D], F32R, tag=f"T3{m}", name=f"T3{m}_{g}")
                nc.scalar.mul(t3, U[m], 0.25)
                T3.append(t3)
            return {"A": A, "T1": T1, "T2": T2, "T3": T3}

        def matvec_step(si, wave, cur, mats, kind, v0c=None):
            """One tail step for all W graphs: u = X @ v (row-form + transpose back).

            Per-graph u rows land in PSUM at 32-aligned partitions (legal PE
            column-group bases), then a strided DMA gathers them to packed rows.
            Returns next v column tiles [128, W] x2."""
            xkey = {0: "T3", 1: "T3", 2: "T3", 3: "T3", 4: "T2", 5: "T1", 6: "A", 7: "A"}[si]
            usb = rows_p.tile([W, D], F32R, tag="usb", name=f"usb_{wave}_{si}")
            for b in range(W):
                X = mats[b][xkey]
                ur = psUR.tile([1, D], F32, tag="ur", name=f"ur_{wave}_{si}_{b}")
                for k in range(2):
                    nc.tensor.matmul(ur, cur[k][:, b:b + 1], X[k],
                                     start=(k == 0), stop=(k == 1))
                # PE can only write PSUM at base partition 0 here, and compute
                # engines cannot write partition b directly: copy to a
                # partition-0 row, then DMA-scatter into the packed row tile.
                us = small_p.tile([1, D], F32R, tag="us", name=f"us_{wave}_{si}_{b}")
                nc.scalar.copy(us, ur)
                if kind == "final":
                    nc.sync.dma_start(out_d[wave * W + b: wave * W + b + 1, :], us.bitcast(F32))
                else:
                    nc.sync.dma_start(usb[b:b + 1, :], us)
            if kind == "final":
                return None
            uc = psT.tile([128, 2 * W], F32, tag="ucols", name=f"uc_{wave}_{si}")
            for m in range(2):
                nc.tensor.matmul(uc[:, m * W:(m + 1) * W],
                                 usb[:, m * 128:(m + 1) * 128], IWr)
            nxt = [tail_p.tile([128, W], F32R, tag=f"VC{m}", name=f"vc_{wave}_{si}_{m}")
                   for m in range(2)]
            for m in range(2):
                ucm = uc[:, m * W:(m + 1) * W]
                if kind == "comb":
                    ucs = tail_p.tile([128, W], F32, tag=f"ucs{m}", name=f"ucs_{wave}_{si}_{m}")
                    nc.scalar.copy(ucs, ucm)
                    nc.vector.scalar_tensor_tensor(nxt[m], cur[m], 3.0, ucs,
                                                   ALU.mult, ALU.subtract)
                elif kind == "a3":
                    # v4 = 3*v0 - 0.25*u
                    a3q = tail_p.tile([128, W], F32R, tag=f"a3q{m}", name=f"a3q_{wave}_{m}")
                    nc.scalar.mul(a3q, ucm, 0.25)
                    nc.vector.scalar_tensor_tensor(nxt[m], v0c[m], 3.0, a3q,
                                                   ALU.mult, ALU.subtract)
            return nxt

        for wave in range(N_WAVES):
            V0ROWS = rows_p.tile([W, D], F32R, tag="v0rows", name=f"v0rows_{wave}")
            mats = []
            for b in range(W):
                g = wave * W + b
                mats.append(phase_a(g, V0ROWS, b))

            # transpose v0 rows -> column tiles [128, W] x2
            v0ps = psT.tile([128, 2 * W], F32, tag="ucols", name=f"v0ps_{wave}")
            for m in range(2):
                nc.tensor.matmul(v0ps[:, m * W:(m + 1) * W],
                                 V0ROWS[:, m * 128:(m + 1) * 128], IWr)
            v0c = []
            for m in range(2):
                v = tail_p.tile([128, W], F32R, tag=f"VC{m}", name=f"v0c_{wave}_{m}")
                nc.scalar.copy(v, v0ps[:, m * W:(m + 1) * W])
                v0c.append(v)

            cur = v0c
            kinds = ["comb", "comb", "a3", "comb", "comb", "comb", "comb", "final"]
            for si in range(8):
                cur = matvec_step(si, wave, cur, mats, kinds[si],
                                  v0c=v0c if kinds[si] == "a3" else None)


_CACHED_NC = None


def _get_nc():
    global _CACHED_NC
    if _CACHED_NC is None:
        _CACHED_NC = build_module()
    return _CACHED_NC


def _run(feat, noise, **spmd_kwargs):
    feat = np.ascontiguousarray(np.asarray(feat), dtype=np.float32)
    noise = np.ascontiguousarray(np.asarray(noise), dtype=np.float32)
    cst, cstb, cstr = _const_arrays()
    nc = _get_nc()
    in_maps = []
    for c in range(N_CORES):
        in_maps.append({
            "feat": feat[c * ROWS_CORE:(c + 1) * ROWS_CORE],
            "noise": noise[c * ROWS_CORE:(c + 1) * ROWS_CORE],
            "cst": cst,
            "cstb": cstb,
            "cstr": cstr,
        })
    return run_bass_kernel_spmd(nc, in_maps, list(range(N_CORES)), **spmd_kwargs)


def kernel(feat, noise, n_per_graph):
    assert int(n_per_graph) == NPG
    try:
        res = _run(feat, noise)
    except Exception:
        # the axon device occasionally reports a transient unrecoverable
        # state; one retry usually succeeds
        res = _run(feat, noise)
    return np.concatenate([res.results[c]["out"] for c in range(N_CORES)], axis=0)



# revision 2
# speedup vs baseline: 6.1548x; 6.1548x over previous
"""DKEPooling Trainium2 kernel, v2 (all-bf16, stage-pipelined).

Per-graph math (d=256, n=512 nodes/graph):
  g  = bf16(feat + 0.01*noise)
  C' = g^T g - s s^T/n            (Gram + rank-1, PSUM accumulated)
  tr = trace(C'); A = C'/tr       (A never materialized: 1/tr folded into evacs)
  V_k' := 1.5I - 0.5 T_k ;  Q_k' = T_k V_k' ;  T_{k+1} = Q_k' V_k'
    (== Newton-Schulz invariant T_{k+1} = 0.25 T_k (3I-T_k)^2, T_0 = A;
     Q_0' = 0.5*A(3I-A) = 0.5*W1 doubles as the final NS factor)
  y  = sqrt(tr/(n-1)) * (1/32) * W1 (3I-T1)(3I-T2)(3I-T3)(3I-T4) * s/n
    tail: 7 matvec steps/graph; (3I-T4)v0 via the 3-matvec Krylov trick.

Scheduling: software pipeline with 9 stages skewed across the 16 graphs so
PE/ACT/DVE streams stay dense (engines execute in program order; emission
order is the schedule).  Tail is batched: 8 graphs' matvec rows land in one
PSUM bank via bf16 quadrant writes (partitions 0/32/64/96 x free halves),
one ACT evac per bank, then a selector-matmul (E4) transposes rows back to
stationary columns - no DMAs inside tail steps.

Sharding: data-parallel over graphs. 8 cores x 16 graphs; no cross-core comm.
"""
import numpy as np

import concourse.bacc as bacc
import concourse.bass as bass
import concourse.mybir as mybir
import concourse.tile as tile
from concourse.bass_utils import run_bass_kernel_spmd

F32 = mybir.dt.float32
BF16 = mybir.dt.bfloat16
ALU = mybir.AluOpType
ACTF = mybir.ActivationFunctionType

N_CORES = 8
D = 256
NPG = 512
B_TOTAL = 128
B_CORE = B_TOTAL // N_CORES      # 16 graphs per core
ROWS_CORE = B_CORE * NPG         # 8192 feat rows per core
WAVE = 8                         # graphs per tail wave
N_WAVES = B_CORE // WAVE

# const tensor (bf16 [128, 646]):
#   [:, 0:512]   C15: 1.5*I_256 in block layout ([1.5I|0 | 0|1.5I])
#                ([:, 128:256] doubles as a zero block)
#   [:, 512:640] I128
#   [:, 640:644] E4 selector (E4[32q, q] = 1)
#   [:, 644:645] ones column
CST_COLS = 646

DEBUG_HOOK = None
MID_HOOK = None


def _const_arrays():
    import ml_dtypes
    cst = np.zeros((128, CST_COLS), np.float32)
    eye = np.eye(128, dtype=np.float32)
    cst[:, 0:128] = 1.5 * eye
    cst[:, 384:512] = 1.5 * eye
    cst[:, 512:640] = eye
    for q in range(4):
        cst[32 * q, 640 + q] = 1.0
    cst[:, 644] = 1.0
    return cst.astype(ml_dtypes.bfloat16)


def build_module():
    nc = bacc.Bacc(None, target_bir_lowering=False)
    feat_d = nc.declare_dram_parameter("feat", [ROWS_CORE, D], F32, isOutput=False)
    noise_d = nc.declare_dram_parameter("noise", [ROWS_CORE, D], F32, isOutput=False)
    cst_d = nc.declare_dram_parameter("cst", [128, CST_COLS], BF16, isOutput=False)
    out_d = nc.declare_dram_parameter("out", [B_CORE, D], F32, isOutput=True)

    with tile.TileContext(nc) as tc:
        _build_tile(tc, nc, feat_d, noise_d, cst_d, out_d)
    nc.compile()
    return nc


def _build_tile(tc, nc, feat_d, noise_d, cst_d, out_d):
    import contextlib
    import concourse.bass_isa as bass_isa
    ctx = contextlib.ExitStack()
    with ctx:
        stage_p = ctx.enter_context(tc.tile_pool(name="stage", bufs=2))
        g_p = ctx.enter_context(tc.tile_pool(name="gp", bufs=3))
        mats_p = ctx.enter_context(tc.tile_pool(name="mats", bufs=B_CORE))
        chain_p = ctx.enter_context(tc.tile_pool(name="chain", bufs=4))
        small_p = ctx.enter_context(tc.tile_pool(name="small", bufs=4))
        tail_p = ctx.enter_context(tc.tile_pool(name="tailp", bufs=4))
        cst_p = ctx.enter_context(tc.tile_pool(name="cstp", bufs=1))
        psG = ctx.enter_context(tc.tile_pool(name="psG", bufs=2, space="PSUM"))
        psC = ctx.enter_context(tc.tile_pool(name="psC", bufs=2, space="PSUM"))
        psS = ctx.enter_context(tc.tile_pool(name="psS", bufs=1, space="PSUM"))
        psR = ctx.enter_context(tc.tile_pool(name="psR", bufs=2, space="PSUM"))
        psV = ctx.enter_context(tc.tile_pool(name="psV", bufs=1, space="PSUM"))

        cst = cst_p.tile([128, CST_COLS], BF16, tag="cst", name="cst_sb")
        nc.gpsimd.dma_start(cst, cst_d[:, :])
        C15 = cst[:, 0:512]
        ZBLK = cst[:, 128:256]
        I128 = cst[:, 512:640]
        E4 = cst[:, 640:644]
        ONES = cst[:, 644:645]

        # zero-init the two tail row banks (quad partitions never written
        # elsewhere must read as exact 0 for the E4 transpose)
        rows_z = []
        for z in range(2):
            rz = psR.tile([128, 512], F32, tag="rows", name=f"rz_{z}")
            nc.tensor.matmul(rz, ZBLK, C15, start=True, stop=True)
            rows_z.append(rz)
        v0rows = []
        for w in range(N_WAVES):
            vr = tail_p.tile([128, 512], BF16, tag="v0rows", name=f"v0rows_{w}")
            nc.vector.memset(vr, 0.0)
            v0rows.append(vr)

        # per-graph state kept across pipeline stages
        ST = [dict() for _ in range(B_CORE)]

        def s_load(g):
            st = ST[g]
            ft = stage_p.tile([128, 4 * D], F32, tag="ft", name=f"ft_{g}")
            nc.gpsimd.dma_start(
                ft, feat_d[g * NPG:(g + 1) * NPG, :].rearrange("(c p) d -> p c d", p=128))
            nz = stage_p.tile([128, 4 * D], F32, tag="nz", name=f"nz_{g}")
            nc.gpsimd.dma_start(
                nz, noise_d[g * NPG:(g + 1) * NPG, :].rearrange("(c p) d -> p c d", p=128))
            st["ft"], st["nz"] = ft, nz

        def s_gb(g):
            st = ST[g]
            gb = g_p.tile([128, 4 * D], BF16, tag="g", name=f"g_{g}")
            nc.vector.scalar_tensor_tensor(gb, st["nz"], 0.01, st["ft"],
                                           ALU.mult, ALU.add)
            st["gb"] = gb

        def s_scol(g):
            st = ST[g]
            gb = st["gb"]
            s_ps = psS.tile([1, D], F32, tag="s", name=f"s_{g}")
            for k in range(4):
                nc.tensor.matmul(s_ps, ONES, gb[:, 256 * k:256 * (k + 1)],
                                 start=(k == 0), stop=(k == 3))
            srow = small_p.tile([1, D], BF16, tag="srow", name=f"srow_{g}")
            nc.scalar.copy(srow, s_ps)
            srow_n = small_p.tile([1, D], BF16, tag="srow_n", name=f"srown_{g}")
            nc.vector.tensor_scalar_mul(srow_n, srow, -1.0 / NPG)
            st["srow"], st["srow_n"] = srow, srow_n

        def s_gram(g):
            # PSUM accumulation groups must be contiguous per bank: emit each
            # m-half's full group (4 gram chunks + rank-1 mean correction)
            # before opening the other half's group.
            st = ST[g]
            gb = st["gb"]
            srow, srow_n = st["srow"], st["srow_n"]
            G = psG.tile([128, 512], F32, tag="G", name=f"G_{g}")
            for m in range(2):
                for k in range(4):
                    nc.tensor.matmul(G[:, 256 * m:256 * (m + 1)],
                                     gb[:, 256 * k + 128 * m:256 * k + 128 * (m + 1)],
                                     gb[:, 256 * k:256 * (k + 1)],
                                     start=(k == 0), stop=False)
                nc.tensor.matmul(G[:, 256 * m:256 * (m + 1)],
                                 srow_n[0:1, 128 * m:128 * (m + 1)], srow,
                                 start=False, stop=True)
            st["G"] = G

        def s_mid(g):
            st = ST[g]
            G = st["G"]
            Gc = chain_p.tile([128, 512], BF16, tag="Gc", name=f"Gc_{g}")
            nc.scalar.copy(Gc, G)
            if MID_HOOK is not None:
                MID_HOOK(g, nc, Gc, G)
            # trace via diag mask + partition all-reduce
            dg = small_p.tile([128, 2], F32, tag="dg", name=f"dg_{g}")
            for m in range(2):
                scr = small_p.tile([128, 128], BF16, tag="scr", name=f"scr_{g}_{m}")
                nc.vector.scalar_tensor_tensor(scr, Gc[:, 384 * m:384 * m + 128],
                                               1.0, I128, ALU.mult, ALU.mult,
                                               accum_out=dg[:, m:m + 1])
            dgs = small_p.tile([128, 1], F32, tag="dgs", name=f"dgs_{g}")
            nc.vector.tensor_add(dgs, dg[:, 0:1], dg[:, 1:2])
            trc = small_p.tile([128, 1], F32, tag="trc", name=f"trc_{g}")
            nc.gpsimd.partition_all_reduce(trc, dgs, 128, bass_isa.ReduceOp.add)
            rcpb = small_p.tile([128, 1], F32, tag="rcpb", name=f"rcpb_{g}")
            nc.vector.reciprocal(rcpb, trc)
            rcpn = small_p.tile([128, 1], F32, tag="rcpn", name=f"rcpn_{g}")
            nc.vector.tensor_scalar_mul(rcpn, rcpb, -0.5)
            cbb = small_p.tile([1, 1], F32, tag="cbb", name=f"cbb_{g}")
            nc.scalar.activation(cbb, trc[0:1, 0:1], ACTF.Sqrt, scale=1.0 / (NPG - 1))
            cb2 = small_p.tile([1, 1], F32, tag="cb2", name=f"cb2_{g}")
            nc.vector.tensor_scalar_mul(cb2, cbb, 1.0 / 8192.0)
            # V0' = 1.5I - 0.5*Gc/tr
            v0p = chain_p.tile([128, 512], BF16, tag="v0p", name=f"v0p_{g}")
            nc.vector.scalar_tensor_tensor(v0p, Gc, rcpn, C15, ALU.mult, ALU.add)
            # v0 row (all tail constants folded): cb2 = sqrt(tr/511)/8192
            v0r = small_p.tile([1, D], BF16, tag="v0r", name=f"v0r_{g}")
            nc.scalar.activation(v0r, st["srow"], ACTF.Copy, scale=cb2)
            q, h = g % 4, (g % WAVE) // 4
            nc.sync.dma_start(
                v0rows[g // WAVE][32 * q:32 * q + 1, 256 * h:256 * h + 256], v0r)
            st["Gc"], st["rcpb"], st["v0p"] = Gc, rcpb, v0p

        def mm256(dst_ps, L, R):
            for m in range(2):
                for k in range(2):
                    nc.tensor.matmul(dst_ps[:, 256 * m:256 * (m + 1)],
                                     L[:, 256 * k + 128 * m:256 * k + 128 * (m + 1)],
                                     R[:, 256 * k:256 * (k + 1)],
                                     start=(k == 0), stop=(k == 1))

        def s_q0(g):
            st = ST[g]
            ps = psC.tile([128, 512], F32, tag="C", name=f"psq0_{g}")
            mm256(ps, st["Gc"], st["v0p"])
            q0 = mats_p.tile([128, 512], BF16, tag="q0", name=f"q0_{g}")
            nc.scalar.activation(q0, ps, ACTF.Copy, scale=st["rcpb"])
            st["q0"] = q0

        def s_t1(g):
            st = ST[g]
            ps = psC.tile([128, 512], F32, tag="C", name=f"pst1_{g}")
            mm256(ps, st["q0"], st["v0p"])
            t1 = mats_p.tile([128, 512], BF16, tag="t1", name=f"t1_{g}")
            nc.scalar.copy(t1, ps)
            v1p = chain_p.tile([128, 512], BF16, tag="v1p", name=f"v1p_{g}")
            nc.vector.scalar_tensor_tensor(v1p, t1, -0.5, C15, ALU.mult, ALU.add)
            st["t1"], st["v1p"] = t1, v1p

        def s_q1(g):
            st = ST[g]
            ps = psC.tile([128, 512], F32, tag="C", name=f"psq1_{g}")
            mm256(ps, st["t1"], st["v1p"])
            qq = chain_p.tile([128, 512], BF16, tag="qq", name=f"qq_{g}")
            nc.scalar.copy(qq, ps)
            st["qq"] = qq

        def s_t2(g):
            st = ST[g]
            ps = psC.tile([128, 512], F32, tag="C", name=f"pst2_{g}")
            mm256(ps, st["qq"], st["v1p"])
            t2 = mats_p.tile([128, 512], BF16, tag="t2", name=f"t2_{g}")
            nc.scalar.copy(t2, ps)
            v2p = chain_p.tile([128, 512], BF16, tag="v2p", name=f"v2p_{g}")
            nc.vector.scalar_tensor_tensor(v2p, t2, -0.5, C15, ALU.mult, ALU.add)
            st["t2"], st["v2p"] = t2, v2p

        def s_q2(g):
            st = ST[g]
            ps = psC.tile([128, 512], F32, tag="C", name=f"psq2_{g}")
            mm256(ps, st["t2"], st["v2p"])
            qx2 = chain_p.tile([128, 512], BF16, tag="qx2", name=f"qx2_{g}")
            nc.scalar.copy(qx2, ps)
            st["qx2"] = qx2

        def s_t3(g):
            st = ST[g]
            ps = psC.tile([128, 512], F32, tag="C", name=f"pst3_{g}")
            mm256(ps, st["qx2"], st["v2p"])
            t3 = mats_p.tile([128, 512], BF16, tag="t3", name=f"t3_{g}")
            nc.scalar.copy(t3, ps)
            st["t3"] = t3

        # ---- phase A+chain: stage-skewed pipeline over the 16 graphs ----
        stages = [s_t3, s_q2, s_t2, s_q1, s_t1, s_q0, s_mid, s_gram, s_scol]
        n_st = len(stages)
        for t in range(B_CORE + n_st + 1):
            if t < B_CORE:
                s_load(t)
            for i, fn in enumerate(stages):
                g = t - (n_st - i)
                if 0 <= g < B_CORE:
                    fn(g)
            if t < B_CORE:
                s_gb(t)

        # ---- batched tail ----
        def transpose_to_cols(src_sb, vc_ps):
            for h in range(2):
                for m in range(2):
                    nc.tensor.matmul(vc_ps[:, 8 * m + 4 * h:8 * m + 4 * h + 4],
                                     src_sb[:, 256 * h + 128 * m:256 * h + 128 * (m + 1)],
                                     E4, start=True, stop=True)

        cur = []
        v0c = []
        vcs = []
        for w in range(N_WAVES):
            vc = psV.tile([128, 16], F32, tag="vc", name=f"v0vc_{w}")
            transpose_to_cols(v0rows[w], vc)
            vcs.append(vc)
        for w in range(N_WAVES):
            v0 = tail_p.tile([128, 16], BF16, tag="v0c", name=f"v0c_{w}")
            nc.scalar.copy(v0, vcs[w])
            v0c.append(v0)
            cur.append(v0)

        xkeys = ["t3", "t3", "t3", "t3", "t2", "t1", "q0"]
        kinds = ["comb", "comb", "a3", "comb", "comb", "comb", "final"]
        snap_v1 = [None, None]
        for si in range(7):
            rows_w = []
            for w in range(N_WAVES):
                rows = psR.tile([128, 512], F32, tag="rows", name=f"rows_{si}_{w}")
                for j in range(WAVE):
                    q, h = j % 4, j // 4
                    X = ST[WAVE * w + j][xkeys[si]]
                    for k in range(2):
                        nc.tensor.matmul(
                            rows[32 * q:32 * q + 1, 256 * h:256 * h + 256],
                            cur[w][:, 8 * k + j:8 * k + j + 1],
                            X[:, 256 * k:256 * (k + 1)],
                            start=(k == 0), stop=(k == 1),
                            tile_position=(0, 32 * q))
                rows_w.append(rows)
            if kinds[si] == "final":
                for w in range(N_WAVES):
                    cf = tail_p.tile([128, 512], F32, tag="cf", name=f"cf_{w}")
                    nc.scalar.copy(cf, rows_w[w])
                    for h in range(2):
                        nc.sync.dma_start(
                            out_d[WAVE * w + 4 * h:WAVE * w + 4 * h + 4, :],
                            cf[0:128:32, 256 * h:256 * h + 256])
                continue
            csb_w = []
            for w in range(N_WAVES):
                csb = tail_p.tile([128, 512], BF16, tag="csb", name=f"csb_{si}_{w}")
                nc.scalar.copy(csb, rows_w[w])
                csb_w.append(csb)
            for w in range(N_WAVES):
                vc = psV.tile([128, 16], F32, tag="vc", name=f"vc_{si}_{w}")
                transpose_to_cols(csb_w[w], vc)
                usb = tail_p.tile([128, 16], BF16, tag="usb", name=f"usb_{si}_{w}")
                if kinds[si] == "a3":
                    nc.scalar.mul(usb, vc, 0.25)
                else:
                    nc.scalar.copy(usb, vc)
                vn = tail_p.tile([128, 16], BF16, tag="vn", name=f"vn_{si}_{w}")
                base = v0c[w] if kinds[si] == "a3" else cur[w]
                nc.vector.scalar_tensor_tensor(vn, base, 3.0, usb,
                                               ALU.mult, ALU.subtract)
                cur[w] = vn
                if si == 0:
                    snap_v1[w] = vn
        if DEBUG_HOOK is not None:
            DEBUG_HOOK(nc, ST, v0rows, snap_v1, v0c)


_CACHED_NC = None


def _get_nc():
    global _CACHED_NC
    if _CACHED_NC is None:
        _CACHED_NC = build_module()
    return _CACHED_NC


def _run(feat, noise, **spmd_kwargs):
    feat = np.ascontiguousarray(np.asarray(feat), dtype=np.float32)
    noise = np.ascontiguousarray(np.asarray(noise), dtype=np.float32)
    cst = _const_arrays()
    nc = _get_nc()
    in_maps = []
    for c in range(N_CORES):
        in_maps.append({
            "feat": feat[c * ROWS_CORE:(c + 1) * ROWS_CORE],
            "noise": noise[c * ROWS_CORE:(c + 1) * ROWS_CORE],
            "cst": cst,
        })
    return run_bass_kernel_spmd(nc, in_maps, list(range(N_CORES)), **spmd_kwargs)


def kernel(feat, noise, n_per_graph):
    assert int(n_per_graph) == NPG
    try:
        res = _run(feat, noise)
    except Exception:
        # the axon device occasionally reports a transient unrecoverable
        # state; one retry usually succeeds
        res = _run(feat, noise)
    return np.concatenate([res.results[c]["out"] for c in range(N_CORES)], axis=0)


# revision 3
# speedup vs baseline: 17.1555x; 2.7873x over previous
"""DKEPooling Trainium2 kernel, v2 (all-bf16, stage-pipelined).

Per-graph math (d=256, n=512 nodes/graph):
  g  = bf16(feat + 0.01*noise)
  C' = g^T g - s s^T/n            (Gram + rank-1, PSUM accumulated)
  tr = trace(C'); A = C'/tr       (A never materialized: 1/tr folded into evacs)
  V_k' := 1.5I - 0.5 T_k ;  Q_k' = T_k V_k' ;  T_{k+1} = Q_k' V_k'
    (== Newton-Schulz invariant T_{k+1} = 0.25 T_k (3I-T_k)^2, T_0 = A;
     Q_0' = 0.5*A(3I-A) = 0.5*W1 doubles as the final NS factor)
  y  = sqrt(tr/(n-1)) * (1/32) * W1 (3I-T1)(3I-T2)(3I-T3)(3I-T4) * s/n
    tail: 7 matvec steps/graph; (3I-T4)v0 via the 3-matvec Krylov trick.

Scheduling: software pipeline with 9 stages skewed across the 16 graphs so
PE/ACT/DVE streams stay dense (engines execute in program order; emission
order is the schedule).  Tail is batched: 8 graphs' matvec rows land in one
PSUM bank via bf16 quadrant writes (partitions 0/32/64/96 x free halves),
one ACT evac per bank, then a selector-matmul (E4) transposes rows back to
stationary columns - no DMAs inside tail steps.

Sharding: data-parallel over graphs. 8 cores x 16 graphs; no cross-core comm.
"""
import numpy as np

import concourse.bacc as bacc
import concourse.bass as bass
import concourse.mybir as mybir
import concourse.tile as tile
from concourse.bass_utils import run_bass_kernel_spmd

F32 = mybir.dt.float32
BF16 = mybir.dt.bfloat16
ALU = mybir.AluOpType
ACTF = mybir.ActivationFunctionType

N_CORES = 8
D = 256
NPG = 512
B_TOTAL = 128
B_CORE = B_TOTAL // N_CORES      # 16 graphs per core
ROWS_CORE = B_CORE * NPG         # 8192 feat rows per core
WAVE = 4                         # graphs per tail wave
N_WAVES = B_CORE // WAVE

# const tensor (bf16 [128, 646]):
#   [:, 0:512]   C15: 1.5*I_256 in block layout ([1.5I|0 | 0|1.5I])
#                ([:, 128:256] doubles as a zero block)
#   [:, 512:640] I128
#   [:, 640:644] E4 selector (E4[32q, q] = 1)
#   [:, 644:645] ones column
CST_COLS = 646

DEBUG_HOOK = None
MID_HOOK = None


def _const_arrays():
    import ml_dtypes
    cst = np.zeros((128, CST_COLS), np.float32)
    eye = np.eye(128, dtype=np.float32)
    cst[:, 0:128] = 1.5 * eye
    cst[:, 384:512] = 1.5 * eye
    cst[:, 512:640] = eye
    for q in range(4):
        cst[32 * q, 640 + q] = 1.0
    cst[:, 644] = 1.0
    return cst.astype(ml_dtypes.bfloat16)


def build_module():
    nc = bacc.Bacc(None, target_bir_lowering=False)
    feat_d = nc.declare_dram_parameter("feat", [ROWS_CORE, D], F32, isOutput=False)
    noise_d = nc.declare_dram_parameter("noise", [ROWS_CORE, D], F32, isOutput=False)
    cst_d = nc.declare_dram_parameter("cst", [128, CST_COLS], BF16, isOutput=False)
    out_d = nc.declare_dram_parameter("out", [B_CORE, D], F32, isOutput=True)

    with tile.TileContext(nc) as tc:
        _build_tile(tc, nc, feat_d, noise_d, cst_d, out_d)
    nc.compile()
    return nc


def _build_tile(tc, nc, feat_d, noise_d, cst_d, out_d):
    import contextlib
    import concourse.bass_isa as bass_isa
    ctx = contextlib.ExitStack()
    with ctx:
        stage_p = ctx.enter_context(tc.tile_pool(name="stage", bufs=2))
        g_p = ctx.enter_context(tc.tile_pool(name="gp", bufs=3))
        mats_p = ctx.enter_context(tc.tile_pool(name="mats", bufs=B_CORE))
        chain_p = ctx.enter_context(tc.tile_pool(name="chain", bufs=4))
        small_p = ctx.enter_context(tc.tile_pool(name="small", bufs=4))
        tail_p = ctx.enter_context(tc.tile_pool(name="tailp", bufs=4))
        vcol_p = ctx.enter_context(tc.tile_pool(name="vcolp", bufs=12))
        cst_p = ctx.enter_context(tc.tile_pool(name="cstp", bufs=1))
        psG = ctx.enter_context(tc.tile_pool(name="psG", bufs=2, space="PSUM"))
        psC = ctx.enter_context(tc.tile_pool(name="psC", bufs=2, space="PSUM"))
        psS = ctx.enter_context(tc.tile_pool(name="psS", bufs=1, space="PSUM"))
        psR = ctx.enter_context(tc.tile_pool(name="psR", bufs=2, space="PSUM"))
        psV = ctx.enter_context(tc.tile_pool(name="psV", bufs=1, space="PSUM"))

        cst = cst_p.tile([128, CST_COLS], BF16, tag="cst", name="cst_sb")
        nc.gpsimd.dma_start(cst, cst_d[:, :])
        C15 = cst[:, 0:512]
        ZBLK = cst[:, 128:256]
        I128 = cst[:, 512:640]
        E4 = cst[:, 640:644]
        ONES = cst[:, 644:645]

        # zero-init the two tail row banks (quad partitions never written
        # elsewhere must read as exact 0 for the E4 transpose)
        rows_z = []
        for z in range(2):
            rz = psR.tile([128, 256], F32, tag="rows", name=f"rz_{z}")
            nc.tensor.matmul(rz, ZBLK, C15[:, 0:256], start=True, stop=True)
            rows_z.append(rz)
        v0rows = []
        for w in range(N_WAVES):
            vr = tail_p.tile([128, 256], BF16, tag="v0rows", name=f"v0rows_{w}")
            nc.vector.memset(vr, 0.0)
            v0rows.append(vr)

        # per-graph state kept across pipeline stages
        ST = [dict() for _ in range(B_CORE)]

        def s_load(g):
            # split across queues: feat on SP HWDGE, noise on gpsimd SWDGE
            st = ST[g]
            ft = stage_p.tile([128, 4 * D], F32, tag="ft", name=f"ft_{g}")
            nc.sync.dma_start(
                ft, feat_d[g * NPG:(g + 1) * NPG, :].rearrange("(c p) d -> p c d", p=128))
            nz = stage_p.tile([128, 4 * D], F32, tag="nz", name=f"nz_{g}")
            nc.gpsimd.dma_start(
                nz, noise_d[g * NPG:(g + 1) * NPG, :].rearrange("(c p) d -> p c d", p=128))
            st["ft"], st["nz"] = ft, nz

        def s_gb(g):
            st = ST[g]
            gb = g_p.tile([128, 4 * D], BF16, tag="g", name=f"g_{g}")
            nc.vector.scalar_tensor_tensor(gb, st["nz"], 0.01, st["ft"],
                                           ALU.mult, ALU.add)
            st["gb"] = gb

        def s_scol(g):
            st = ST[g]
            gb = st["gb"]
            s_ps = psS.tile([1, D], F32, tag="s", name=f"s_{g}")
            for k in range(4):
                nc.tensor.matmul(s_ps, ONES, gb[:, 256 * k:256 * (k + 1)],
                                 start=(k == 0), stop=(k == 3))
            srow = small_p.tile([1, D], BF16, tag="srow", name=f"srow_{g}")
            nc.scalar.copy(srow, s_ps)
            srow_n = small_p.tile([1, D], BF16, tag="srow_n", name=f"srown_{g}")
            nc.vector.tensor_scalar_mul(srow_n, srow, -1.0 / NPG)
            st["srow"], st["srow_n"] = srow, srow_n

        def s_gram(g):
            # PSUM accumulation groups must be contiguous per bank: emit each
            # m-half's full group (4 gram chunks + rank-1 mean correction)
            # before opening the other half's group.
            st = ST[g]
            gb = st["gb"]
            srow, srow_n = st["srow"], st["srow_n"]
            G = psG.tile([128, 512], F32, tag="G", name=f"G_{g}")
            for m in range(2):
                for k in range(4):
                    nc.tensor.matmul(G[:, 256 * m:256 * (m + 1)],
                                     gb[:, 256 * k + 128 * m:256 * k + 128 * (m + 1)],
                                     gb[:, 256 * k:256 * (k + 1)],
                                     start=(k == 0), stop=False)
                nc.tensor.matmul(G[:, 256 * m:256 * (m + 1)],
                                 srow_n[0:1, 128 * m:128 * (m + 1)], srow,
                                 start=False, stop=True)
            st["G"] = G

        def s_mid(g):
            st = ST[g]
            G = st["G"]
            Gc = chain_p.tile([128, 512], BF16, tag="Gc", name=f"Gc_{g}")
            nc.scalar.copy(Gc, G)
            if MID_HOOK is not None:
                MID_HOOK(g, nc, Gc, G)
            # trace via diag mask + partition all-reduce
            dg = small_p.tile([128, 2], F32, tag="dg", name=f"dg_{g}")
            for m in range(2):
                scr = small_p.tile([128, 128], BF16, tag="scr", name=f"scr_{g}_{m}")
                nc.vector.scalar_tensor_tensor(scr, Gc[:, 384 * m:384 * m + 128],
                                               1.0, I128, ALU.mult, ALU.mult,
                                               accum_out=dg[:, m:m + 1])
            dgs = small_p.tile([128, 1], F32, tag="dgs", name=f"dgs_{g}")
            nc.vector.tensor_add(dgs, dg[:, 0:1], dg[:, 1:2])
            trc = small_p.tile([128, 1], F32, tag="trc", name=f"trc_{g}")
            nc.gpsimd.partition_all_reduce(trc, dgs, 128, bass_isa.ReduceOp.add)
            rcpb = small_p.tile([128, 1], F32, tag="rcpb", name=f"rcpb_{g}")
            nc.vector.reciprocal(rcpb, trc)
            rcpn = small_p.tile([128, 1], F32, tag="rcpn", name=f"rcpn_{g}")
            nc.vector.tensor_scalar_mul(rcpn, rcpb, -0.5)
            cbb = small_p.tile([1, 1], F32, tag="cbb", name=f"cbb_{g}")
            nc.scalar.activation(cbb, trc[0:1, 0:1], ACTF.Sqrt, scale=1.0 / (NPG - 1))
            cb2 = small_p.tile([1, 1], F32, tag="cb2", name=f"cb2_{g}")
            nc.vector.tensor_scalar_mul(cb2, cbb, 1.0 / 8192.0)
            # V0' = 1.5I - 0.5*Gc/tr
            v0p = chain_p.tile([128, 512], BF16, tag="v0p", name=f"v0p_{g}")
            nc.vector.scalar_tensor_tensor(v0p, Gc, rcpn, C15, ALU.mult, ALU.add)
            # v0 row (all tail constants folded): cb2 = sqrt(tr/511)/8192
            v0r = small_p.tile([1, D], BF16, tag="v0r", name=f"v0r_{g}")
            nc.scalar.activation(v0r, st["srow"], ACTF.Copy, scale=cb2)
            nc.sync.dma_start(
                v0rows[g // WAVE][32 * (g % WAVE):32 * (g % WAVE) + 1, :], v0r)
            st["Gc"], st["rcpb"], st["v0p"] = Gc, rcpb, v0p

        def mm256(dst_ps, L, R):
            for m in range(2):
                for k in range(2):
                    nc.tensor.matmul(dst_ps[:, 256 * m:256 * (m + 1)],
                                     L[:, 256 * k + 128 * m:256 * k + 128 * (m + 1)],
                                     R[:, 256 * k:256 * (k + 1)],
                                     start=(k == 0), stop=(k == 1))

        def s_q0(g):
            st = ST[g]
            ps = psC.tile([128, 512], F32, tag="C", name=f"psq0_{g}")
            mm256(ps, st["Gc"], st["v0p"])
            q0 = mats_p.tile([128, 512], BF16, tag="q0", name=f"q0_{g}")
            nc.scalar.activation(q0, ps, ACTF.Copy, scale=st["rcpb"])
            st["q0"] = q0

        def s_t1(g):
            st = ST[g]
            ps = psC.tile([128, 512], F32, tag="C", name=f"pst1_{g}")
            mm256(ps, st["q0"], st["v0p"])
            t1 = mats_p.tile([128, 512], BF16, tag="t1", name=f"t1_{g}")
            nc.scalar.copy(t1, ps)
            v1p = chain_p.tile([128, 512], BF16, tag="v1p", name=f"v1p_{g}")
            nc.vector.scalar_tensor_tensor(v1p, t1, -0.5, C15, ALU.mult, ALU.add)
            st["t1"], st["v1p"] = t1, v1p

        def s_q1(g):
            st = ST[g]
            ps = psC.tile([128, 512], F32, tag="C", name=f"psq1_{g}")
            mm256(ps, st["t1"], st["v1p"])
            qq = chain_p.tile([128, 512], BF16, tag="qq", name=f"qq_{g}")
            nc.vector.tensor_scalar_mul(qq, ps, 1.0)
            st["qq"] = qq

        def s_t2(g):
            st = ST[g]
            ps = psC.tile([128, 512], F32, tag="C", name=f"pst2_{g}")
            mm256(ps, st["qq"], st["v1p"])
            t2 = mats_p.tile([128, 512], BF16, tag="t2", name=f"t2_{g}")
            nc.scalar.copy(t2, ps)
            v2p = chain_p.tile([128, 512], BF16, tag="v2p", name=f"v2p_{g}")
            nc.vector.scalar_tensor_tensor(v2p, t2, -0.5, C15, ALU.mult, ALU.add)
            st["t2"], st["v2p"] = t2, v2p

        def s_q2(g):
            st = ST[g]
            ps = psC.tile([128, 512], F32, tag="C", name=f"psq2_{g}")
            mm256(ps, st["t2"], st["v2p"])
            qx2 = chain_p.tile([128, 512], BF16, tag="qx2", name=f"qx2_{g}")
            nc.scalar.copy(qx2, ps)
            st["qx2"] = qx2

        def s_t3(g):
            st = ST[g]
            ps = psC.tile([128, 512], F32, tag="C", name=f"pst3_{g}")
            mm256(ps, st["qx2"], st["v2p"])
            t3 = mats_p.tile([128, 512], BF16, tag="t3", name=f"t3_{g}")
            nc.scalar.copy(t3, ps)
            st["t3"] = t3

        # ---- batched tail, tick-scheduled to overlap late phase-A ----
        # wave w = graphs 4w..4w+3; per step: 8 matvec matmuls land the 4
        # u-rows at quad partitions of one PSUM bank (part1), then a
        # selector transpose + combine produces the next v columns (part2).
        xkeys = ["t3", "t3", "t3", "t3", "t2", "t1", "q0"]
        kinds = ["comb", "comb", "a3", "comb", "comb", "comb", "final"]
        cur = {}
        v0c = {}
        TMP = {}

        def transpose_to_cols(src_sb, vc_ps):
            for m in range(2):
                nc.tensor.matmul(vc_ps[:, 4 * m:4 * m + 4],
                                 src_sb[:, 128 * m:128 * (m + 1)],
                                 E4, start=True, stop=True)

        def emit_v0c(w):
            def f():
                vc = psV.tile([128, 8], F32, tag="vc", name=f"v0vc_{w}")
                transpose_to_cols(v0rows[w], vc)
                v0 = vcol_p.tile([128, 8], BF16, tag="v0c", name=f"v0c_{w}")
                nc.scalar.copy(v0, vc)
                v0c[w] = v0
                cur[w] = v0
            return f

        def emit_p1(si, w):
            def f():
                rows = psR.tile([128, 256], F32, tag="rows", name=f"rows_{si}_{w}")
                for j in range(WAVE):
                    X = ST[WAVE * w + j][xkeys[si]]
                    for k in range(2):
                        nc.tensor.matmul(
                            rows[32 * j:32 * j + 1, :],
                            cur[w][:, 4 * k + j:4 * k + j + 1],
                            X[:, 256 * k:256 * (k + 1)],
                            start=(k == 0), stop=(k == 1),
                            tile_position=(0, 32 * j))
                if kinds[si] == "final":
                    cf = tail_p.tile([128, 256], F32, tag="cf", name=f"cf_{w}")
                    nc.scalar.copy(cf, rows)
                    nc.sync.dma_start(out_d[WAVE * w:WAVE * (w + 1), :],
                                      cf[0:128:32, :])
                else:
                    csb = tail_p.tile([128, 256], BF16, tag="csb",
                                      name=f"csb_{si}_{w}")
                    nc.vector.tensor_scalar_mul(csb, rows, 1.0)
                    TMP[(si, w)] = csb
            return f

        def emit_p2(si, w):
            def f():
                csb = TMP.pop((si, w))
                vc = psV.tile([128, 8], F32, tag="vc", name=f"vc_{si}_{w}")
                transpose_to_cols(csb, vc)
                vn = vcol_p.tile([128, 8], BF16, tag="vn", name=f"vn_{si}_{w}")
                if kinds[si] == "a3":
                    usb = tail_p.tile([128, 8], BF16, tag="usb",
                                      name=f"usb_{si}_{w}")
                    nc.scalar.mul(usb, vc, 0.25)
                    nc.vector.scalar_tensor_tensor(vn, v0c[w], 3.0, usb,
                                                   ALU.mult, ALU.subtract)
                else:
                    # combine straight from the transpose PSUM on DVE
                    nc.vector.scalar_tensor_tensor(vn, cur[w], 3.0, vc,
                                                   ALU.mult, ALU.subtract)
                cur[w] = vn
            return f

        from collections import defaultdict
        tail_sched = defaultdict(list)
        for w in range(N_WAVES):
            t0 = 13 + WAVE * w
            tail_sched[t0 - 1].append(emit_v0c(w))
            for si in range(7):
                tail_sched[t0 + si].append(emit_p1(si, w))
                if kinds[si] != "final":
                    tail_sched[t0 + si + 1].insert(0, emit_p2(si, w))

        # ---- phase A+chain: stage-skewed pipeline over the 16 graphs,
        #      with tail ticks interleaved ----
        stages = [s_t3, s_q2, s_t2, s_q1, s_t1, s_q0, s_mid, s_gram, s_scol]
        n_st = len(stages)
        n_ticks = max(B_CORE + n_st + 1, max(tail_sched) + 1)
        for t in range(n_ticks):
            if t < B_CORE:
                s_load(t)
            for i, fn in enumerate(stages):
                g = t - (n_st - i)
                if 0 <= g < B_CORE:
                    fn(g)
            if t < B_CORE:
                s_gb(t)
            for f in tail_sched.get(t, ()):
                f()


_CACHED_NC = None


def _get_nc():
    global _CACHED_NC
    if _CACHED_NC is None:
        _CACHED_NC = build_module()
    return _CACHED_NC


def _run(feat, noise, **spmd_kwargs):
    feat = np.ascontiguousarray(np.asarray(feat), dtype=np.float32)
    noise = np.ascontiguousarray(np.asarray(noise), dtype=np.float32)
    cst = _const_arrays()
    nc = _get_nc()
    in_maps = []
    for c in range(N_CORES):
        in_maps.append({
            "feat": feat[c * ROWS_CORE:(c + 1) * ROWS_CORE],
            "noise": noise[c * ROWS_CORE:(c + 1) * ROWS_CORE],
            "cst": cst,
        })
    return run_bass_kernel_spmd(nc, in_maps, list(range(N_CORES)), **spmd_kwargs)


def kernel(feat, noise, n_per_graph):
    assert int(n_per_graph) == NPG
    try:
        res = _run(feat, noise)
    except Exception:
        # the axon device occasionally reports a transient unrecoverable
        # state; one retry usually succeeds
        res = _run(feat, noise)
    return np.concatenate([res.results[c]["out"] for c in range(N_CORES)], axis=0)


# revision 4
# speedup vs baseline: 18.7645x; 1.0938x over previous
"""DKEPooling Trainium2 kernel, v2 (all-bf16, stage-pipelined).

Per-graph math (d=256, n=512 nodes/graph):
  g  = bf16(feat + 0.01*noise)
  C' = g^T g - s s^T/n            (Gram + rank-1, PSUM accumulated)
  tr = trace(C'); A = C'/tr       (A never materialized: 1/tr folded into evacs)
  V_k' := 1.5I - 0.5 T_k ;  Q_k' = T_k V_k' ;  T_{k+1} = Q_k' V_k'
    (== Newton-Schulz invariant T_{k+1} = 0.25 T_k (3I-T_k)^2, T_0 = A;
     Q_0' = 0.5*A(3I-A) = 0.5*W1 doubles as the final NS factor)
  y  = sqrt(tr/(n-1)) * (1/32) * W1 (3I-T1)(3I-T2)(3I-T3)(3I-T4) * s/n
    tail: 7 matvec steps/graph; (3I-T4)v0 via the 3-matvec Krylov trick.

Scheduling: software pipeline with 9 stages skewed across the 16 graphs so
PE/ACT/DVE streams stay dense (engines execute in program order; emission
order is the schedule).  Tail is batched: 8 graphs' matvec rows land in one
PSUM bank via bf16 quadrant writes (partitions 0/32/64/96 x free halves),
one ACT evac per bank, then a selector-matmul (E4) transposes rows back to
stationary columns - no DMAs inside tail steps.

Sharding: data-parallel over graphs. 8 cores x 16 graphs; no cross-core comm.
"""
import numpy as np

import concourse.bacc as bacc
import concourse.bass as bass
import concourse.mybir as mybir
import concourse.tile as tile
from concourse.bass_utils import run_bass_kernel_spmd

F32 = mybir.dt.float32
BF16 = mybir.dt.bfloat16
ALU = mybir.AluOpType
ACTF = mybir.ActivationFunctionType

N_CORES = 8
D = 256
NPG = 512
B_TOTAL = 128
B_CORE = B_TOTAL // N_CORES      # 16 graphs per core
ROWS_CORE = B_CORE * NPG         # 8192 feat rows per core
WAVE = 4                         # graphs per tail wave
N_WAVES = B_CORE // WAVE

# const tensor (bf16 [128, 646]):
#   [:, 0:512]   C15: 1.5*I_256 in block layout ([1.5I|0 | 0|1.5I])
#                ([:, 128:256] doubles as a zero block)
#   [:, 512:640] I128
#   [:, 640:644] E4 selector (E4[32q, q] = 1)
#   [:, 644:645] ones column
CST_COLS = 646

DEBUG_HOOK = None
MID_HOOK = None


def _const_arrays():
    import ml_dtypes
    cst = np.zeros((128, CST_COLS), np.float32)
    eye = np.eye(128, dtype=np.float32)
    cst[:, 0:128] = 1.5 * eye
    cst[:, 384:512] = 1.5 * eye
    cst[:, 512:640] = eye
    for q in range(4):
        cst[32 * q, 640 + q] = 1.0
    cst[:, 644] = 1.0
    return cst.astype(ml_dtypes.bfloat16)


def build_module():
    nc = bacc.Bacc(None, target_bir_lowering=False)
    feat_d = nc.declare_dram_parameter("feat", [ROWS_CORE, D], F32, isOutput=False)
    noise_d = nc.declare_dram_parameter("noise", [ROWS_CORE, D], F32, isOutput=False)
    cst_d = nc.declare_dram_parameter("cst", [128, CST_COLS], BF16, isOutput=False)
    out_d = nc.declare_dram_parameter("out", [B_CORE, D], F32, isOutput=True)

    with tile.TileContext(nc) as tc:
        _build_tile(tc, nc, feat_d, noise_d, cst_d, out_d)
    nc.compile()
    return nc


def _build_tile(tc, nc, feat_d, noise_d, cst_d, out_d):
    import contextlib
    import concourse.bass_isa as bass_isa
    ctx = contextlib.ExitStack()
    with ctx:
        stage_p = ctx.enter_context(tc.tile_pool(name="stage", bufs=2))
        g_p = ctx.enter_context(tc.tile_pool(name="gp", bufs=3))
        mats_p = ctx.enter_context(tc.tile_pool(name="mats", bufs=B_CORE))
        chain_p = ctx.enter_context(tc.tile_pool(name="chain", bufs=4))
        small_p = ctx.enter_context(tc.tile_pool(name="small", bufs=4))
        tail_p = ctx.enter_context(tc.tile_pool(name="tailp", bufs=4))
        vcol_p = ctx.enter_context(tc.tile_pool(name="vcolp", bufs=12))
        cst_p = ctx.enter_context(tc.tile_pool(name="cstp", bufs=1))
        psG = ctx.enter_context(tc.tile_pool(name="psG", bufs=2, space="PSUM"))
        psC = ctx.enter_context(tc.tile_pool(name="psC", bufs=2, space="PSUM"))
        psS = ctx.enter_context(tc.tile_pool(name="psS", bufs=1, space="PSUM"))
        psR = ctx.enter_context(tc.tile_pool(name="psR", bufs=1, space="PSUM"))
        psV = ctx.enter_context(tc.tile_pool(name="psV", bufs=2, space="PSUM"))

        cst = cst_p.tile([128, CST_COLS], BF16, tag="cst", name="cst_sb")
        nc.gpsimd.dma_start(cst, cst_d[:, :])
        C15 = cst[:, 0:512]
        ZBLK = cst[:, 128:256]
        I128 = cst[:, 512:640]
        E4 = cst[:, 640:644]
        ONES = cst[:, 644:645]

        v0rows = []
        for w in range(N_WAVES):
            vr = tail_p.tile([128, 256], BF16, tag="v0rows", name=f"v0rows_{w}")
            nc.vector.memset(vr, 0.0)
            v0rows.append(vr)

        # per-graph state kept across pipeline stages
        ST = [dict() for _ in range(B_CORE)]

        def s_load(g):
            # split across queues: feat on SP HWDGE, noise on gpsimd SWDGE
            st = ST[g]
            ft = stage_p.tile([128, 4 * D], F32, tag="ft", name=f"ft_{g}")
            nc.sync.dma_start(
                ft, feat_d[g * NPG:(g + 1) * NPG, :].rearrange("(c p) d -> p c d", p=128))
            nz = stage_p.tile([128, 4 * D], F32, tag="nz", name=f"nz_{g}")
            nc.gpsimd.dma_start(
                nz, noise_d[g * NPG:(g + 1) * NPG, :].rearrange("(c p) d -> p c d", p=128))
            st["ft"], st["nz"] = ft, nz

        def s_gb(g):
            st = ST[g]
            gb = g_p.tile([128, 4 * D], BF16, tag="g", name=f"g_{g}")
            nc.vector.scalar_tensor_tensor(gb, st["nz"], 0.01, st["ft"],
                                           ALU.mult, ALU.add)
            st["gb"] = gb

        def s_scol(g):
            st = ST[g]
            gb = st["gb"]
            s_ps = psS.tile([1, D], F32, tag="s", name=f"s_{g}")
            for k in range(4):
                nc.tensor.matmul(s_ps, ONES, gb[:, 256 * k:256 * (k + 1)],
                                 start=(k == 0), stop=(k == 3))
            srow = small_p.tile([1, D], BF16, tag="srow", name=f"srow_{g}")
            nc.scalar.copy(srow, s_ps)
            srow_n = small_p.tile([1, D], BF16, tag="srow_n", name=f"srown_{g}")
            nc.vector.tensor_scalar_mul(srow_n, srow, -1.0 / NPG)
            st["srow"], st["srow_n"] = srow, srow_n

        def s_gram(g):
            # PSUM accumulation groups must be contiguous per bank: emit each
            # m-half's full group (4 gram chunks + rank-1 mean correction)
            # before opening the other half's group.
            st = ST[g]
            gb = st["gb"]
            srow, srow_n = st["srow"], st["srow_n"]
            G = psG.tile([128, 512], F32, tag="G", name=f"G_{g}")
            for m in range(2):
                for k in range(4):
                    nc.tensor.matmul(G[:, 256 * m:256 * (m + 1)],
                                     gb[:, 256 * k + 128 * m:256 * k + 128 * (m + 1)],
                                     gb[:, 256 * k:256 * (k + 1)],
                                     start=(k == 0), stop=False)
                nc.tensor.matmul(G[:, 256 * m:256 * (m + 1)],
                                 srow_n[0:1, 128 * m:128 * (m + 1)], srow,
                                 start=False, stop=True)
            st["G"] = G

        def s_mid(g):
            st = ST[g]
            G = st["G"]
            Gc = chain_p.tile([128, 512], BF16, tag="Gc", name=f"Gc_{g}")
            nc.scalar.copy(Gc, G)
            if MID_HOOK is not None:
                MID_HOOK(g, nc, Gc, G)
            # trace via diag mask + partition all-reduce
            dg = small_p.tile([128, 2], F32, tag="dg", name=f"dg_{g}")
            for m in range(2):
                scr = small_p.tile([128, 128], BF16, tag="scr", name=f"scr_{g}_{m}")
                nc.vector.scalar_tensor_tensor(scr, Gc[:, 384 * m:384 * m + 128],
                                               1.0, I128, ALU.mult, ALU.mult,
                                               accum_out=dg[:, m:m + 1])
            dgs = small_p.tile([128, 1], F32, tag="dgs", name=f"dgs_{g}")
            nc.vector.tensor_add(dgs, dg[:, 0:1], dg[:, 1:2])
            trc = small_p.tile([128, 1], F32, tag="trc", name=f"trc_{g}")
            nc.gpsimd.partition_all_reduce(trc, dgs, 128, bass_isa.ReduceOp.add)
            rcpb = small_p.tile([128, 1], F32, tag="rcpb", name=f"rcpb_{g}")
            nc.vector.reciprocal(rcpb, trc)
            rcpn = small_p.tile([128, 1], F32, tag="rcpn", name=f"rcpn_{g}")
            nc.vector.tensor_scalar_mul(rcpn, rcpb, -0.5)
            cbb = small_p.tile([1, 1], F32, tag="cbb", name=f"cbb_{g}")
            nc.scalar.activation(cbb, trc[0:1, 0:1], ACTF.Sqrt, scale=1.0 / (NPG - 1))
            cb2 = small_p.tile([1, 1], F32, tag="cb2", name=f"cb2_{g}")
            nc.vector.tensor_scalar_mul(cb2, cbb, 1.0 / 8192.0)
            # V0' = 1.5I - 0.5*Gc/tr
            v0p = chain_p.tile([128, 512], BF16, tag="v0p", name=f"v0p_{g}")
            nc.vector.scalar_tensor_tensor(v0p, Gc, rcpn, C15, ALU.mult, ALU.add)
            # v0 row (all tail constants folded): cb2 = sqrt(tr/511)/8192
            v0r = small_p.tile([1, D], BF16, tag="v0r", name=f"v0r_{g}")
            nc.scalar.activation(v0r, st["srow"], ACTF.Copy, scale=cb2)
            nc.sync.dma_start(
                v0rows[g // WAVE][32 * (g % WAVE):32 * (g % WAVE) + 1, :], v0r)
            st["Gc"], st["rcpb"], st["v0p"] = Gc, rcpb, v0p

        def mm256(dst_ps, L, R):
            for m in range(2):
                for k in range(2):
                    nc.tensor.matmul(dst_ps[:, 256 * m:256 * (m + 1)],
                                     L[:, 256 * k + 128 * m:256 * k + 128 * (m + 1)],
                                     R[:, 256 * k:256 * (k + 1)],
                                     start=(k == 0), stop=(k == 1))

        def s_q0(g):
            st = ST[g]
            ps = psC.tile([128, 512], F32, tag="C", name=f"psq0_{g}")
            mm256(ps, st["Gc"], st["v0p"])
            q0 = mats_p.tile([128, 512], BF16, tag="q0", name=f"q0_{g}")
            nc.scalar.activation(q0, ps, ACTF.Copy, scale=st["rcpb"])
            st["q0"] = q0

        def s_t1(g):
            st = ST[g]
            ps = psC.tile([128, 512], F32, tag="C", name=f"pst1_{g}")
            mm256(ps, st["q0"], st["v0p"])
            t1 = mats_p.tile([128, 512], BF16, tag="t1", name=f"t1_{g}")
            nc.scalar.copy(t1, ps)
            v1p = chain_p.tile([128, 512], BF16, tag="v1p", name=f"v1p_{g}")
            nc.vector.scalar_tensor_tensor(v1p, t1, -0.5, C15, ALU.mult, ALU.add)
            st["t1"], st["v1p"] = t1, v1p

        def s_q1(g):
            st = ST[g]
            ps = psC.tile([128, 512], F32, tag="C", name=f"psq1_{g}")
            mm256(ps, st["t1"], st["v1p"])
            qq = chain_p.tile([128, 512], BF16, tag="qq", name=f"qq_{g}")
            nc.vector.tensor_scalar_mul(qq, ps, 1.0)
            st["qq"] = qq

        def s_t2(g):
            st = ST[g]
            ps = psC.tile([128, 512], F32, tag="C", name=f"pst2_{g}")
            mm256(ps, st["qq"], st["v1p"])
            t2 = mats_p.tile([128, 512], BF16, tag="t2", name=f"t2_{g}")
            nc.scalar.copy(t2, ps)
            v2p = chain_p.tile([128, 512], BF16, tag="v2p", name=f"v2p_{g}")
            nc.vector.scalar_tensor_tensor(v2p, t2, -0.5, C15, ALU.mult, ALU.add)
            st["t2"], st["v2p"] = t2, v2p

        def s_q2(g):
            st = ST[g]
            ps = psC.tile([128, 512], F32, tag="C", name=f"psq2_{g}")
            mm256(ps, st["t2"], st["v2p"])
            qx2 = chain_p.tile([128, 512], BF16, tag="qx2", name=f"qx2_{g}")
            nc.scalar.copy(qx2, ps)
            st["qx2"] = qx2

        def s_t3(g):
            st = ST[g]
            ps = psC.tile([128, 512], F32, tag="C", name=f"pst3_{g}")
            mm256(ps, st["qx2"], st["v2p"])
            t3 = mats_p.tile([128, 512], BF16, tag="t3", name=f"t3_{g}")
            nc.scalar.copy(t3, ps)
            st["t3"] = t3

        # ---- batched tail, tick-scheduled to overlap late phase-A ----
        # wave w = graphs 4w..4w+3; per step: 8 matvec matmuls land the 4
        # u-rows at quad partitions of one PSUM bank (part1), then a
        # selector transpose + combine produces the next v columns (part2).
        xkeys = ["t3", "t3", "t3", "t3", "t2", "t1", "q0"]
        kinds = ["comb", "comb", "a3", "comb", "comb", "comb", "final"]
        cur = {}
        v0c = {}
        TMP = {}

        def transpose_to_cols(src_sb, vc_ps):
            for m in range(2):
                nc.tensor.matmul(vc_ps[:, 4 * m:4 * m + 4],
                                 src_sb[:, 128 * m:128 * (m + 1)],
                                 E4, start=True, stop=True)

        def emit_v0c(w):
            def f():
                vc = psV.tile([128, 8], F32, tag="vc", name=f"v0vc_{w}")
                transpose_to_cols(v0rows[w], vc)
                v0 = vcol_p.tile([128, 8], BF16, tag="v0c", name=f"v0c_{w}")
                nc.scalar.copy(v0, vc)
                v0c[w] = v0
                cur[w] = v0
            return f

        def emit_p1(si, w):
            # Column-form matvec: stationary = X block [128,128], moving = v
            # column -> u chunks land directly as PSUM columns (no rows bank,
            # no transpose).  Final step stays row-form for the output DMA.
            def f():
                if kinds[si] == "final":
                    rows = psR.tile([128, 256], F32, tag="rows",
                                    name=f"rows_{si}_{w}")
                    for j in range(WAVE):
                        X = ST[WAVE * w + j][xkeys[si]]
                        for k in range(2):
                            nc.tensor.matmul(
                                rows[32 * j:32 * j + 1, :],
                                cur[w][:, 4 * k + j:4 * k + j + 1],
                                X[:, 256 * k:256 * (k + 1)],
                                start=(k == 0), stop=(k == 1),
                                tile_position=(0, 32 * j))
                    cf = tail_p.tile([128, 256], F32, tag="cf", name=f"cf_{w}")
                    nc.scalar.copy(cf, rows)
                    nc.sync.dma_start(out_d[WAVE * w:WAVE * (w + 1), :],
                                      cf[0:128:32, :])
                    return
                vc = psV.tile([128, 8], F32, tag="vc", name=f"vc_{si}_{w}")
                for j in range(WAVE):
                    X = ST[WAVE * w + j][xkeys[si]]
                    for m in range(2):
                        for k in range(2):
                            nc.tensor.matmul(
                                vc[:, 4 * m + j:4 * m + j + 1],
                                X[:, 256 * k + 128 * m:256 * k + 128 * (m + 1)],
                                cur[w][:, 4 * k + j:4 * k + j + 1],
                                start=(k == 0), stop=(k == 1))
                TMP[(si, w)] = vc
            return f

        def emit_p2(si, w):
            def f():
                vc = TMP.pop((si, w))
                vn = vcol_p.tile([128, 8], BF16, tag="vn", name=f"vn_{si}_{w}")
                if kinds[si] == "a3":
                    usb = tail_p.tile([128, 8], BF16, tag="usb",
                                      name=f"usb_{si}_{w}")
                    nc.scalar.mul(usb, vc, 0.25)
                    nc.vector.scalar_tensor_tensor(vn, v0c[w], 3.0, usb,
                                                   ALU.mult, ALU.subtract)
                else:
                    # combine straight from the matvec PSUM columns on DVE
                    nc.vector.scalar_tensor_tensor(vn, cur[w], 3.0, vc,
                                                   ALU.mult, ALU.subtract)
                cur[w] = vn
            return f

        from collections import defaultdict
        tail_sched = defaultdict(list)
        for w in range(N_WAVES):
            t0 = 13 + WAVE * w
            tail_sched[t0 - 1].append(emit_v0c(w))
            for si in range(7):
                tail_sched[t0 + si].append(emit_p1(si, w))
                if kinds[si] != "final":
                    tail_sched[t0 + si + 1].insert(0, emit_p2(si, w))

        # ---- phase A+chain: stage-skewed pipeline over the 16 graphs,
        #      with tail ticks interleaved ----
        stages = [s_t3, s_q2, s_t2, s_q1, s_t1, s_q0, s_mid, s_gram, s_scol]
        n_st = len(stages)
        n_ticks = max(B_CORE + n_st + 1, max(tail_sched) + 1)
        for t in range(n_ticks):
            if t < B_CORE:
                s_load(t)
            for i, fn in enumerate(stages):
                g = t - (n_st - i)
                if 0 <= g < B_CORE:
                    fn(g)
            if t < B_CORE:
                s_gb(t)
            for f in tail_sched.get(t, ()):
                f()


_CACHED_NC = None


def _get_nc():
    global _CACHED_NC
    if _CACHED_NC is None:
        _CACHED_NC = build_module()
    return _CACHED_NC


def _run(feat, noise, **spmd_kwargs):
    feat = np.ascontiguousarray(np.asarray(feat), dtype=np.float32)
    noise = np.ascontiguousarray(np.asarray(noise), dtype=np.float32)
    cst = _const_arrays()
    nc = _get_nc()
    in_maps = []
    for c in range(N_CORES):
        in_maps.append({
            "feat": feat[c * ROWS_CORE:(c + 1) * ROWS_CORE],
            "noise": noise[c * ROWS_CORE:(c + 1) * ROWS_CORE],
            "cst": cst,
        })
    return run_bass_kernel_spmd(nc, in_maps, list(range(N_CORES)), **spmd_kwargs)


def kernel(feat, noise, n_per_graph):
    assert int(n_per_graph) == NPG
    try:
        res = _run(feat, noise)
    except Exception:
        # the axon device occasionally reports a transient unrecoverable
        # state; one retry usually succeeds
        res = _run(feat, noise)
    return np.concatenate([res.results[c]["out"] for c in range(N_CORES)], axis=0)


# revision 5
# speedup vs baseline: 29.3931x; 1.5664x over previous
"""DKEPooling Trainium2 kernel, v2 (all-bf16, stage-pipelined).

Per-graph math (d=256, n=512 nodes/graph):
  g  = bf16(feat + 0.01*noise)
  C' = g^T g - s s^T/n            (Gram + rank-1, PSUM accumulated)
  tr = trace(C'); A = C'/tr       (A never materialized: 1/tr folded into evacs)
  V_k' := 1.5I - 0.5 T_k ;  Q_k' = T_k V_k' ;  T_{k+1} = Q_k' V_k'
    (== Newton-Schulz invariant T_{k+1} = 0.25 T_k (3I-T_k)^2, T_0 = A;
     Q_0' = 0.5*A(3I-A) = 0.5*W1 doubles as the final NS factor)
  y  = sqrt(tr/(n-1)) * (1/32) * W1 (3I-T1)(3I-T2)(3I-T3)(3I-T4) * s/n
    tail: 7 matvec steps/graph; (3I-T4)v0 via the 3-matvec Krylov trick.

Scheduling: software pipeline with 9 stages skewed across the 16 graphs so
PE/ACT/DVE streams stay dense (engines execute in program order; emission
order is the schedule); PSUM evacuations are load-balanced between ACT and
DVE (DVE PSUM reads work on this runtime).  Tail (4-graph waves, ticks
interleaved into late phase-A): matvecs run column-form - stationary = X
block, moving = v column - so u lands directly as PSUM columns and the
combine (3v - u) reads PSUM on DVE; only v0 needs a row->column transpose
(bf16 quadrant rows + E4 selector matmul) and only the final step uses
row-form for the output DMA.  No DMAs inside tail steps.

Sharding: data-parallel over graphs. 8 cores x 16 graphs; no cross-core comm.
"""
import numpy as np

import concourse.bacc as bacc
import concourse.bass as bass
import concourse.mybir as mybir
import concourse.tile as tile
from concourse.bass_utils import run_bass_kernel_spmd

F32 = mybir.dt.float32
BF16 = mybir.dt.bfloat16
ALU = mybir.AluOpType
ACTF = mybir.ActivationFunctionType

N_CORES = 8
D = 256
NPG = 512
B_TOTAL = 128
B_CORE = B_TOTAL // N_CORES      # 16 graphs per core
ROWS_CORE = B_CORE * NPG         # 8192 feat rows per core
WAVE = 4                         # graphs per tail wave
N_WAVES = B_CORE // WAVE

# const tensor (bf16 [128, 646]):
#   [:, 0:512]   C15: 1.5*I_256 in block layout ([1.5I|0 | 0|1.5I])
#                ([:, 128:256] doubles as a zero block)
#   [:, 512:640] I128
#   [:, 640:644] E4 selector (E4[32q, q] = 1)
#   [:, 644:645] ones column
CST_COLS = 646

DEBUG_HOOK = None
MID_HOOK = None


def _const_arrays():
    import ml_dtypes
    cst = np.zeros((128, CST_COLS), np.float32)
    eye = np.eye(128, dtype=np.float32)
    cst[:, 0:128] = 1.5 * eye
    cst[:, 384:512] = 1.5 * eye
    cst[:, 512:640] = eye
    for q in range(4):
        cst[32 * q, 640 + q] = 1.0
    cst[:, 644] = 1.0
    return cst.astype(ml_dtypes.bfloat16)


def build_module():
    nc = bacc.Bacc(None, target_bir_lowering=False)
    feat_d = nc.declare_dram_parameter("feat", [ROWS_CORE, D], F32, isOutput=False)
    noise_d = nc.declare_dram_parameter("noise", [ROWS_CORE, D], F32, isOutput=False)
    cst_d = nc.declare_dram_parameter("cst", [128, CST_COLS], BF16, isOutput=False)
    out_d = nc.declare_dram_parameter("out", [B_CORE, D], F32, isOutput=True)

    with tile.TileContext(nc) as tc:
        _build_tile(tc, nc, feat_d, noise_d, cst_d, out_d)
    nc.compile()
    return nc


def _build_tile(tc, nc, feat_d, noise_d, cst_d, out_d):
    import contextlib
    import concourse.bass_isa as bass_isa
    ctx = contextlib.ExitStack()
    with ctx:
        stage_p = ctx.enter_context(tc.tile_pool(name="stage", bufs=2))
        g_p = ctx.enter_context(tc.tile_pool(name="gp", bufs=3))
        mats_p = ctx.enter_context(tc.tile_pool(name="mats", bufs=B_CORE))
        chain_p = ctx.enter_context(tc.tile_pool(name="chain", bufs=4))
        small_p = ctx.enter_context(tc.tile_pool(name="small", bufs=4))
        tail_p = ctx.enter_context(tc.tile_pool(name="tailp", bufs=4))
        vcol_p = ctx.enter_context(tc.tile_pool(name="vcolp", bufs=12))
        cst_p = ctx.enter_context(tc.tile_pool(name="cstp", bufs=1))
        psG = ctx.enter_context(tc.tile_pool(name="psG", bufs=2, space="PSUM"))
        psC = ctx.enter_context(tc.tile_pool(name="psC", bufs=2, space="PSUM"))
        psS = ctx.enter_context(tc.tile_pool(name="psS", bufs=1, space="PSUM"))
        psR = ctx.enter_context(tc.tile_pool(name="psR", bufs=1, space="PSUM"))
        psV = ctx.enter_context(tc.tile_pool(name="psV", bufs=2, space="PSUM"))

        cst = cst_p.tile([128, CST_COLS], BF16, tag="cst", name="cst_sb")
        nc.gpsimd.dma_start(cst, cst_d[:, :])
        C15 = cst[:, 0:512]
        ZBLK = cst[:, 128:256]
        I128 = cst[:, 512:640]
        E4 = cst[:, 640:644]
        ONES = cst[:, 644:645]

        v0rows = []
        for w in range(N_WAVES):
            vr = tail_p.tile([128, 256], BF16, tag="v0rows", name=f"v0rows_{w}")
            nc.vector.memset(vr, 0.0)
            v0rows.append(vr)

        # per-graph state kept across pipeline stages
        ST = [dict() for _ in range(B_CORE)]

        def s_load(g):
            # split across queues: feat on SP HWDGE, noise on gpsimd SWDGE
            st = ST[g]
            ft = stage_p.tile([128, 4 * D], F32, tag="ft", name=f"ft_{g}")
            nc.sync.dma_start(
                ft, feat_d[g * NPG:(g + 1) * NPG, :].rearrange("(c p) d -> p c d", p=128))
            nz = stage_p.tile([128, 4 * D], F32, tag="nz", name=f"nz_{g}")
            nc.gpsimd.dma_start(
                nz, noise_d[g * NPG:(g + 1) * NPG, :].rearrange("(c p) d -> p c d", p=128))
            st["ft"], st["nz"] = ft, nz

        def s_gb(g):
            st = ST[g]
            gb = g_p.tile([128, 4 * D], BF16, tag="g", name=f"g_{g}")
            nc.vector.scalar_tensor_tensor(gb, st["nz"], 0.01, st["ft"],
                                           ALU.mult, ALU.add)
            st["gb"] = gb

        def s_scol(g):
            st = ST[g]
            gb = st["gb"]
            s_ps = psS.tile([1, D], F32, tag="s", name=f"s_{g}")
            for k in range(4):
                nc.tensor.matmul(s_ps, ONES, gb[:, 256 * k:256 * (k + 1)],
                                 start=(k == 0), stop=(k == 3))
            srow = small_p.tile([1, D], BF16, tag="srow", name=f"srow_{g}")
            nc.scalar.copy(srow, s_ps)
            srow_n = small_p.tile([1, D], BF16, tag="srow_n", name=f"srown_{g}")
            nc.vector.tensor_scalar_mul(srow_n, srow, -1.0 / NPG)
            st["srow"], st["srow_n"] = srow, srow_n

        def s_gram(g):
            # PSUM accumulation groups must be contiguous per bank: emit each
            # m-half's full group (4 gram chunks + rank-1 mean correction)
            # before opening the other half's group.
            st = ST[g]
            gb = st["gb"]
            srow, srow_n = st["srow"], st["srow_n"]
            G = psG.tile([128, 512], F32, tag="G", name=f"G_{g}")
            for m in range(2):
                for k in range(4):
                    nc.tensor.matmul(G[:, 256 * m:256 * (m + 1)],
                                     gb[:, 256 * k + 128 * m:256 * k + 128 * (m + 1)],
                                     gb[:, 256 * k:256 * (k + 1)],
                                     start=(k == 0), stop=False)
                nc.tensor.matmul(G[:, 256 * m:256 * (m + 1)],
                                 srow_n[0:1, 128 * m:128 * (m + 1)], srow,
                                 start=False, stop=True)
            st["G"] = G

        def s_mid(g):
            st = ST[g]
            G = st["G"]
            Gc = chain_p.tile([128, 512], BF16, tag="Gc", name=f"Gc_{g}")
            nc.scalar.copy(Gc, G)
            if MID_HOOK is not None:
                MID_HOOK(g, nc, Gc, G)
            # trace via diag mask + partition all-reduce
            dg = small_p.tile([128, 2], F32, tag="dg", name=f"dg_{g}")
            for m in range(2):
                scr = small_p.tile([128, 128], BF16, tag="scr", name=f"scr_{g}_{m}")
                nc.vector.scalar_tensor_tensor(scr, Gc[:, 384 * m:384 * m + 128],
                                               1.0, I128, ALU.mult, ALU.mult,
                                               accum_out=dg[:, m:m + 1])
            dgs = small_p.tile([128, 1], F32, tag="dgs", name=f"dgs_{g}")
            nc.vector.tensor_add(dgs, dg[:, 0:1], dg[:, 1:2])
            trc = small_p.tile([128, 1], F32, tag="trc", name=f"trc_{g}")
            nc.gpsimd.partition_all_reduce(trc, dgs, 128, bass_isa.ReduceOp.add)
            rcpb = small_p.tile([128, 1], F32, tag="rcpb", name=f"rcpb_{g}")
            nc.vector.reciprocal(rcpb, trc)
            rcpn = small_p.tile([128, 1], F32, tag="rcpn", name=f"rcpn_{g}")
            nc.vector.tensor_scalar_mul(rcpn, rcpb, -0.5)
            cbb = small_p.tile([1, 1], F32, tag="cbb", name=f"cbb_{g}")
            nc.scalar.activation(cbb, trc[0:1, 0:1], ACTF.Sqrt, scale=1.0 / (NPG - 1))
            cb2 = small_p.tile([1, 1], F32, tag="cb2", name=f"cb2_{g}")
            nc.vector.tensor_scalar_mul(cb2, cbb, 1.0 / 8192.0)
            # V0' = 1.5I - 0.5*Gc/tr
            v0p = chain_p.tile([128, 512], BF16, tag="v0p", name=f"v0p_{g}")
            nc.vector.scalar_tensor_tensor(v0p, Gc, rcpn, C15, ALU.mult, ALU.add)
            # v0 row (all tail constants folded): cb2 = sqrt(tr/511)/8192
            v0r = small_p.tile([1, D], BF16, tag="v0r", name=f"v0r_{g}")
            nc.scalar.activation(v0r, st["srow"], ACTF.Copy, scale=cb2)
            nc.sync.dma_start(
                v0rows[g // WAVE][32 * (g % WAVE):32 * (g % WAVE) + 1, :], v0r)
            st["Gc"], st["rcpb"], st["v0p"] = Gc, rcpb, v0p

        def mm256(dst_ps, L, R):
            for m in range(2):
                for k in range(2):
                    nc.tensor.matmul(dst_ps[:, 256 * m:256 * (m + 1)],
                                     L[:, 256 * k + 128 * m:256 * k + 128 * (m + 1)],
                                     R[:, 256 * k:256 * (k + 1)],
                                     start=(k == 0), stop=(k == 1))

        def s_q0(g):
            st = ST[g]
            ps = psC.tile([128, 512], F32, tag="C", name=f"psq0_{g}")
            mm256(ps, st["Gc"], st["v0p"])
            q0 = mats_p.tile([128, 512], BF16, tag="q0", name=f"q0_{g}")
            nc.scalar.activation(q0, ps, ACTF.Copy, scale=st["rcpb"])
            st["q0"] = q0

        def s_t1(g):
            st = ST[g]
            ps = psC.tile([128, 512], F32, tag="C", name=f"pst1_{g}")
            mm256(ps, st["q0"], st["v0p"])
            t1 = mats_p.tile([128, 512], BF16, tag="t1", name=f"t1_{g}")
            nc.scalar.copy(t1, ps)
            v1p = chain_p.tile([128, 512], BF16, tag="v1p", name=f"v1p_{g}")
            nc.vector.scalar_tensor_tensor(v1p, t1, -0.5, C15, ALU.mult, ALU.add)
            st["t1"], st["v1p"] = t1, v1p

        def s_q1(g):
            st = ST[g]
            ps = psC.tile([128, 512], F32, tag="C", name=f"psq1_{g}")
            mm256(ps, st["t1"], st["v1p"])
            qq = chain_p.tile([128, 512], BF16, tag="qq", name=f"qq_{g}")
            nc.vector.tensor_scalar_mul(qq, ps, 1.0)
            st["qq"] = qq

        def s_t2(g):
            st = ST[g]
            ps = psC.tile([128, 512], F32, tag="C", name=f"pst2_{g}")
            mm256(ps, st["qq"], st["v1p"])
            t2 = mats_p.tile([128, 512], BF16, tag="t2", name=f"t2_{g}")
            nc.scalar.copy(t2, ps)
            v2p = chain_p.tile([128, 512], BF16, tag="v2p", name=f"v2p_{g}")
            nc.vector.scalar_tensor_tensor(v2p, t2, -0.5, C15, ALU.mult, ALU.add)
            st["t2"], st["v2p"] = t2, v2p

        def s_q2(g):
            st = ST[g]
            ps = psC.tile([128, 512], F32, tag="C", name=f"psq2_{g}")
            mm256(ps, st["t2"], st["v2p"])
            qx2 = chain_p.tile([128, 512], BF16, tag="qx2", name=f"qx2_{g}")
            nc.scalar.copy(qx2, ps)
            st["qx2"] = qx2

        def s_t3(g):
            st = ST[g]
            ps = psC.tile([128, 512], F32, tag="C", name=f"pst3_{g}")
            mm256(ps, st["qx2"], st["v2p"])
            t3 = mats_p.tile([128, 512], BF16, tag="t3", name=f"t3_{g}")
            nc.scalar.copy(t3, ps)
            st["t3"] = t3

        # ---- batched tail, tick-scheduled to overlap late phase-A ----
        # wave w = graphs 4w..4w+3; per step: 8 matvec matmuls land the 4
        # u-rows at quad partitions of one PSUM bank (part1), then a
        # selector transpose + combine produces the next v columns (part2).
        xkeys = ["t3", "t3", "t3", "t3", "t2", "t1", "q0"]
        kinds = ["comb", "comb", "a3", "comb", "comb", "comb", "final"]
        cur = {}
        v0c = {}
        TMP = {}

        def transpose_to_cols(src_sb, vc_ps):
            for m in range(2):
                nc.tensor.matmul(vc_ps[:, 4 * m:4 * m + 4],
                                 src_sb[:, 128 * m:128 * (m + 1)],
                                 E4, start=True, stop=True)

        def emit_v0c(w):
            def f():
                vc = psV.tile([128, 8], F32, tag="vc", name=f"v0vc_{w}")
                transpose_to_cols(v0rows[w], vc)
                v0 = vcol_p.tile([128, 8], BF16, tag="v0c", name=f"v0c_{w}")
                nc.scalar.copy(v0, vc)
                v0c[w] = v0
                cur[w] = v0
            return f

        def emit_p1(si, w):
            # Column-form matvec: stationary = X block [128,128], moving = v
            # column -> u chunks land directly as PSUM columns (no rows bank,
            # no transpose).  Final step stays row-form for the output DMA.
            def f():
                if kinds[si] == "final":
                    rows = psR.tile([128, 256], F32, tag="rows",
                                    name=f"rows_{si}_{w}")
                    for j in range(WAVE):
                        X = ST[WAVE * w + j][xkeys[si]]
                        for k in range(2):
                            nc.tensor.matmul(
                                rows[32 * j:32 * j + 1, :],
                                cur[w][:, 4 * k + j:4 * k + j + 1],
                                X[:, 256 * k:256 * (k + 1)],
                                start=(k == 0), stop=(k == 1),
                                tile_position=(0, 32 * j))
                    cf = tail_p.tile([128, 256], F32, tag="cf", name=f"cf_{w}")
                    nc.scalar.copy(cf, rows)
                    nc.sync.dma_start(out_d[WAVE * w:WAVE * (w + 1), :],
                                      cf[0:128:32, :])
                    return
                vc = psV.tile([128, 8], F32, tag="vc", name=f"vc_{si}_{w}")
                for j in range(WAVE):
                    X = ST[WAVE * w + j][xkeys[si]]
                    for m in range(2):
                        for k in range(2):
                            nc.tensor.matmul(
                                vc[:, 4 * m + j:4 * m + j + 1],
                                X[:, 256 * k + 128 * m:256 * k + 128 * (m + 1)],
                                cur[w][:, 4 * k + j:4 * k + j + 1],
                                start=(k == 0), stop=(k == 1))
                TMP[(si, w)] = vc
            return f

        def emit_p2(si, w):
            def f():
                vc = TMP.pop((si, w))
                vn = vcol_p.tile([128, 8], BF16, tag="vn", name=f"vn_{si}_{w}")
                if kinds[si] == "a3":
                    usb = tail_p.tile([128, 8], BF16, tag="usb",
                                      name=f"usb_{si}_{w}")
                    nc.scalar.mul(usb, vc, 0.25)
                    nc.vector.scalar_tensor_tensor(vn, v0c[w], 3.0, usb,
                                                   ALU.mult, ALU.subtract)
                else:
                    # combine straight from the matvec PSUM columns on DVE
                    nc.vector.scalar_tensor_tensor(vn, cur[w], 3.0, vc,
                                                   ALU.mult, ALU.subtract)
                cur[w] = vn
            return f

        from collections import defaultdict
        tail_sched = defaultdict(list)
        for w in range(N_WAVES):
            t0 = 13 + WAVE * w
            tail_sched[t0 - 1].append(emit_v0c(w))
            for si in range(7):
                tail_sched[t0 + si].append(emit_p1(si, w))
                if kinds[si] != "final":
                    tail_sched[t0 + si + 1].insert(0, emit_p2(si, w))

        # ---- phase A+chain: stage-skewed pipeline over the 16 graphs,
        #      with tail ticks interleaved ----
        stages = [s_t3, s_q2, s_t2, s_q1, s_t1, s_q0, s_mid, s_gram, s_scol]
        n_st = len(stages)
        n_ticks = max(B_CORE + n_st + 1, max(tail_sched) + 1)
        for t in range(n_ticks):
            if t < B_CORE:
                s_load(t)
            for i, fn in enumerate(stages):
                g = t - (n_st - i)
                if 0 <= g < B_CORE:
                    fn(g)
            if t < B_CORE:
                s_gb(t)
            for f in tail_sched.get(t, ()):
                f()


_CACHED_NC = None


def _get_nc():
    global _CACHED_NC
    if _CACHED_NC is None:
        _CACHED_NC = build_module()
    return _CACHED_NC


def _run(feat, noise, **spmd_kwargs):
    feat = np.ascontiguousarray(np.asarray(feat), dtype=np.float32)
    noise = np.ascontiguousarray(np.asarray(noise), dtype=np.float32)
    cst = _const_arrays()
    nc = _get_nc()
    in_maps = []
    for c in range(N_CORES):
        in_maps.append({
            "feat": feat[c * ROWS_CORE:(c + 1) * ROWS_CORE],
            "noise": noise[c * ROWS_CORE:(c + 1) * ROWS_CORE],
            "cst": cst,
        })
    return run_bass_kernel_spmd(nc, in_maps, list(range(N_CORES)), **spmd_kwargs)


def kernel(feat, noise, n_per_graph):
    assert int(n_per_graph) == NPG
    try:
        res = _run(feat, noise)
    except Exception:
        # the axon device occasionally reports a transient unrecoverable
        # state; one retry usually succeeds
        res = _run(feat, noise)
    return np.concatenate([res.results[c]["out"] for c in range(N_CORES)], axis=0)
